# revision 1
# baseline (speedup 1.0000x reference)
"""Trainium2 Bass kernel for Linformer-style sparse attention.

Problem shapes (hardcoded): B=4, S=4096, D=1024, H=16, HD=64, LK=256.

Sharding (8 cores): core c -> (batch b = c//2, sequence half = c%2).
Each core:
  - computes Q/K/V for its 2048 rows (all heads),
  - computes partial [Kp^T; Vp^T] = (K|V)^T @ E^T over its rows,
  - pair AllReduce ([0,1],[2,3],[4,5],[6,7]) completes Kp/Vp (1 MiB bf16),
  - attention (softmax over LK=256) + output projection for its own rows,
  - writes its [2048, 1024] slice of the output directly (no final collective).

All matmuls run in bf16 (f32->bf16 casts happen inside SWDGE DMAs); X^T and
E^T are produced with the XBAR dma_start_transpose (single HWDGE ring - two
concurrent transpose rings corrupt data). Q^T matmul chunks are interleaved
into the E^T-transpose-bound partials window to keep the PE dense, and the
AllReduce is covered by the tail Q chunk + readbacks.
"""

import sys

sys.path.insert(0, "/opt/trn_rl_repo")

from contextlib import ExitStack

import numpy as np

from concourse import bacc, bass_utils, mybir, tile
from concourse.masks import make_identity

B, S, D = 4, 4096, 1024
H, HD, LK = 16, 64, 256
SL = S // 2            # local sequence rows per core
P = 128
NSC = SL // P          # 16 s-chunks of 128
NDC = D // P           # 8 d-chunks of 128
NSN = SL // 512        # 4 s-chunks of 512
f32 = mybir.dt.float32
bf16 = mybir.dt.bfloat16
PAIRS = [[0, 1], [2, 3], [4, 5], [6, 7]]


def _build(include_biases: bool, debug: bool = False):
    nc = bacc.Bacc("TRN2", target_bir_lowering=False, num_devices=8)

    X_e = nc.declare_dram_parameter("X", [SL, D], f32, isOutput=False)
    mask_e = nc.declare_dram_parameter("mask", [SL], f32, isOutput=False)
    Wq_e = nc.declare_dram_parameter("Wq", [D, D], f32, isOutput=False)
    bq_e = nc.declare_dram_parameter("bq", [D], f32, isOutput=False)
    Wk_e = nc.declare_dram_parameter("Wk", [D, D], f32, isOutput=False)
    bk_e = nc.declare_dram_parameter("bk", [D], f32, isOutput=False)
    Wv_e = nc.declare_dram_parameter("Wv", [D, D], f32, isOutput=False)
    bv_e = nc.declare_dram_parameter("bv", [D], f32, isOutput=False)
    E_e = nc.declare_dram_parameter("E", [H, LK, SL], f32, isOutput=False)
    Wo_e = nc.declare_dram_parameter("Wo", [D, D], f32, isOutput=False)
    bo_e = nc.declare_dram_parameter("bo", [D], f32, isOutput=False)
    out_e = nc.declare_dram_parameter("out", [SL, D], f32, isOutput=True)

    ebf_d = nc.dram_tensor("ebf", [H, LK, SL], bf16, kind="Internal")
    # AllReduce bounce (bf16): per head [KpT ; VpT] stacked [128, 256] flat
    cc_in = nc.dram_tensor("cc_in", [H, P * LK], bf16, kind="Internal")
    cc_out = nc.dram_tensor("cc_out", [H, P * LK], bf16, kind="Internal")

    with tile.TileContext(nc) as tc:
        ctx = ExitStack()
        with ctx:
            const_pool = ctx.enter_context(tc.tile_pool(name="consts", bufs=1))

            # ---------------- constants ----------------
            m_sb = const_pool.tile([P, NSC], f32, name="m_sb")
            nc.sync.dma_start(m_sb[:], mask_e.ap().rearrange("(o p) -> p o", p=P))
            bq_sb = const_pool.tile([P, NDC], f32, name="bq_sb")
            nc.sync.dma_start(bq_sb[:], bq_e.ap().rearrange("(o p) -> p o", p=P))
            bo_bc = const_pool.tile([P, D], bf16, name="bo_bc")
            nc.gpsimd.dma_start(out=bo_bc[:], in_=bo_e.ap()[None, :].to_broadcast((P, D)))
            if include_biases:
                bkv_bc = const_pool.tile([P, 2, D], f32, name="bkv_bc")
                nc.sync.dma_start(bkv_bc[:, 0, :], bk_e.ap()[None, :].to_broadcast((P, D)))
                nc.sync.dma_start(bkv_bc[:, 1, :], bv_e.ap()[None, :].to_broadcast((P, D)))
            id_sb = const_pool.tile([P, P], bf16, name="id_sb")
            make_identity(nc, id_sb[:])
            ones_sb = const_pool.tile([P, HD], bf16, name="ones_sb")
            nc.vector.memset(ones_sb[:], 1.0)

            # ---------------- X^T: cast to SBUF, XBAR-transpose per s-chunk ----------------
            # xT layout: [d_in(P), sc(16), dc(8), s_in(128)]
            xT, free_xT = tc.tile([P, NSC, NDC, P], bf16, name="xT")
            xstage, free_xstage = tc.tile([P, NSC, D], bf16, name="xstage")
            nc.gpsimd.dma_start(
                out=xstage[:], in_=X_e.ap().rearrange("(o p) n -> p o n", p=P)
            )
            for scg in range(4):
                nc.sync.dma_start_transpose(
                    xT[:, scg * 4 : (scg + 1) * 4, :, :],
                    xstage[:, scg * 4 : (scg + 1) * 4, :],
                )
            free_xstage()

            qT_lo, free_qT_lo = tc.tile([P, 4, SL], bf16, name="qT_lo")
            wq_bf, free_wq = tc.tile([P, NDC, D], bf16, name="wq_bf")
            kv, free_kv = tc.tile([P, NSC, H, 2, HD], bf16, name="kv")
            eT_scope = ExitStack()
            eT_pool = eT_scope.enter_context(tc.tile_pool(name="eT", bufs=5))
            part_pool = eT_scope.enter_context(tc.tile_pool(name="part", bufs=3))
            wk_bf, free_wk = tc.tile([P, NDC, D], bf16, name="wk_bf")
            wv_bf, free_wv = tc.tile([P, NDC, D], bf16, name="wv_bf")
            nc.gpsimd.dma_start(
                out=wk_bf[:], in_=Wk_e.ap().rearrange("(o p) n -> p o n", p=P)
            )
            nc.gpsimd.dma_start(
                out=wv_bf[:], in_=Wv_e.ap().rearrange("(o p) n -> p o n", p=P)
            )
            nc.gpsimd.dma_start(
                out=wq_bf[:], in_=Wq_e.ap().rearrange("(o p) n -> p o n", p=P)
            )

            # ---------------- E: cast to DRAM scratch, per-head XBAR-transpose ----------------
            for h in range(H):
                nc.gpsimd.dma_start(out=ebf_d[h], in_=E_e[h])
            # eT layout per head: [s_in(P), so(16), k(256)]
            eT_tiles = {}

            def stage_eT(h):
                eT = eT_pool.tile([P, NSC, LK], bf16, name="eT")
                nc.sync.dma_start_transpose(eT[:], ebf_d[h])
                eT_tiles[h] = eT

            for h in range(5):
                stage_eT(h)

            with (
                tc.tile_pool(name="ps_kvq", bufs=4, space="PSUM") as ps_kvq,
                tc.tile_pool(name="ps_part", bufs=2, space="PSUM") as ps_part,
            ):
                # ---------------- K/V natural [s, dh] (masked, bf16) ----------------
                for sc in range(NSC):
                    for t, w_bf in ((0, wk_bf), (1, wv_bf)):
                        for half in range(2):
                            ps = ps_kvq.tile([P, 512], f32, name="ps_kv", tag="mm512")
                            for dc in range(NDC):
                                nc.tensor.matmul(
                                    ps[:],
                                    xT[:, sc, dc, :],
                                    w_bf[:, dc, half * 512 : (half + 1) * 512],
                                    start=(dc == 0),
                                    stop=(dc == NDC - 1),
                                )
                            if include_biases:
                                nc.vector.tensor_tensor(
                                    out=ps[:],
                                    in0=ps[:],
                                    in1=bkv_bc[:, t, half * 512 : (half + 1) * 512],
                                    op=mybir.AluOpType.add,
                                )
                            nc.vector.tensor_scalar(
                                out=kv[:, sc, half * 8 : (half + 1) * 8, t, :],
                                in0=ps[:],
                                scalar1=m_sb[:, sc : sc + 1],
                                scalar2=None,
                                op0=mybir.AluOpType.mult,
                            )

                def q_chunk(mc, dst, dj):
                    for sn in range(NSN):
                        ps = ps_kvq.tile([P, 512], f32, name="psq", tag="mm512")
                        for dc in range(NDC):
                            nc.tensor.matmul(
                                ps[:],
                                wq_bf[:, dc, mc * P : (mc + 1) * P],
                                xT[:, sn * 4 : (sn + 1) * 4, dc, :],
                                start=(dc == 0),
                                stop=(dc == NDC - 1),
                            )
                        nc.vector.tensor_scalar(
                            out=dst[:, mc - dj, sn * 512 : (sn + 1) * 512],
                            in0=ps[:],
                            scalar1=bq_sb[:, mc : mc + 1],
                            scalar2=None,
                            op0=mybir.AluOpType.add,
                        )

                # ---------------- partial [KpT; VpT] per head (Q low chunks woven in) ----------------
                for h in range(H):
                    if h in (5, 8, 11, 14):
                        q_chunk((h - 5) // 3, qT_lo, 0)
                    eT = eT_tiles.pop(h)
                    kp_ps = ps_part.tile([P, LK], f32, name="kp_ps")
                    for so in range(NSC):
                        nc.tensor.matmul(
                            kp_ps[:],
                            kv[:, so, h, :, :],
                            eT[:, so, :],
                            start=(so == 0),
                            stop=(so == NSC - 1),
                        )
                    kp_sb = part_pool.tile([P, LK], bf16, name="kp_sb")
                    nc.vector.tensor_copy(kp_sb[:], kp_ps[:])
                    nc.gpsimd.dma_start(
                        out=cc_in[h].rearrange("(a b) -> a b", a=P),
                        in_=kp_sb[:],
                    )
                    if h + 5 < H:
                        stage_eT(h + 5)

                # ---------------- AllReduce over pairs (bf16, 1 MiB) ----------------
                nc.gpsimd.collective_compute(
                    "AllReduce",
                    mybir.AluOpType.add,
                    replica_groups=PAIRS,
                    ins=[cc_in[:].opt()],
                    outs=[cc_out[:].opt()],
                )

            free_wv()
            free_wk()
            eT_scope.close()
            free_kv()


            # ---------------- Q^T high half (covers the AllReduce) ----------------
            qT_hi, free_qT_hi = tc.tile([P, 4, SL], bf16, name="qT_hi")
            with tc.tile_pool(name="ps_q", bufs=4, space="PSUM") as ps_q:
                for mc in range(4, NDC):
                    for sn in range(NSN):
                        ps = ps_q.tile([P, 512], f32, name="psq2")
                        for dc in range(NDC):
                            nc.tensor.matmul(
                                ps[:],
                                wq_bf[:, dc, mc * P : (mc + 1) * P],
                                xT[:, sn * 4 : (sn + 1) * 4, dc, :],
                                start=(dc == 0),
                                stop=(dc == NDC - 1),
                            )
                        nc.vector.tensor_scalar(
                            out=qT_hi[:, mc - 4, sn * 512 : (sn + 1) * 512],
                            in0=ps[:],
                            scalar1=bq_sb[:, mc : mc + 1],
                            scalar2=None,
                            op0=mybir.AluOpType.add,
                        )

            # ---------------- read back reduced Kp^T / Vp ----------------
            kpT, free_kpT = tc.tile([P, H // 2, LK], bf16, name="kpT")
            vp_sb2, free_vp = tc.tile([P, H, 2, HD], bf16, name="vp_sb2")
            with (
                tc.tile_pool(name="vpT_pool", bufs=3) as vpT_pool,
                tc.tile_pool(name="ps_tp", bufs=3, space="PSUM") as ps_tp,
            ):
                for h in range(H):
                    par = (h % 2) * 64
                    nc.gpsimd.dma_start(
                        out=kpT[par : par + 64, h // 2, :],
                        in_=cc_out[h, 0 : 64 * LK].rearrange("(a b) -> a b", a=64),
                    )
                    vpT_sb = vpT_pool.tile([64, 2, P], bf16, name="vpT_sb")
                    nc.gpsimd.dma_start(
                        out=vpT_sb[:],
                        in_=cc_out[h, 64 * LK :].rearrange("(a b) -> a b", a=64),
                    )
                    for c in range(2):
                        tp_ps = ps_tp.tile([P, HD], bf16, name="tp_ps")
                        nc.tensor.transpose(
                            tp_ps[:], vpT_sb[:, c, :], id_sb[0:64, 0:64]
                        )
                        nc.vector.tensor_copy(vp_sb2[:, h, c, :], tp_ps[:])

            # ---------------- attention (sn outer) + inline output projection ----------------
            xoT, free_xoT = tc.tile([P, NDC, SL], bf16, name="xoT")
            wo_bf, free_wo = tc.tile([P, NDC, D], bf16, name="wo_bf")
            nc.gpsimd.dma_start(
                out=wo_bf[:], in_=Wo_e.ap().rearrange("(o p) n -> p o n", p=P)
            )
            with (
                tc.tile_pool(name="at_pool", bufs=3) as at_pool,
                tc.tile_pool(name="rbc_pool", bufs=2) as rbc_pool,
                tc.tile_pool(name="osb_pool", bufs=3) as osb_pool,
                tc.tile_pool(name="ps_dot", bufs=2, space="PSUM") as ps_dot,
                tc.tile_pool(name="ps_xoden", bufs=4, space="PSUM") as ps_xoden,
                tc.tile_pool(name="ps_out", bufs=2, space="PSUM") as ps_out,
            ):
                def attn_pair(sn, j):
                    # heads (2j, 2j+1): even parity on partitions 0-63, odd on 64-127
                    ssl = slice(sn * 512, (sn + 1) * 512)
                    ats = []
                    for par in (0, 64):
                        at = at_pool.tile([P, 2, 512], bf16, name="at")
                        for kc in range(2):
                            dps = ps_dot.tile([P, 512], f32, name="dps")
                            qsrc = qT_lo if j < 4 else qT_hi
                            nc.tensor.matmul(
                                dps[:],
                                kpT[par : par + 64, j, kc * P : (kc + 1) * P],
                                qsrc[par : par + 64, j % 4, ssl],
                                start=True,
                                stop=True,
                            )
                            nc.scalar.activation(
                                out=at[:, kc, :],
                                in_=dps[:],
                                func=mybir.ActivationFunctionType.Exp,
                                scale=0.125,
                            )
                        ats.append(at)
                    xo_ps = ps_xoden.tile([P, 512], f32, name="xo_ps", tag="xoden")
                    den_ps = ps_xoden.tile([P, 512], f32, name="den_ps", tag="xoden")
                    for kc in range(2):
                        for pi, par in ((0, 0), (1, 64)):
                            h = 2 * j + pi
                            nc.tensor.matmul(
                                xo_ps[par : par + 64, :],
                                vp_sb2[:, h, kc, :],
                                ats[pi][:, kc, :],
                                start=(kc == 0),
                                stop=(kc == 1),
                                skip_group_check=True,
                            )
                            nc.tensor.matmul(
                                den_ps[par : par + 64, :],
                                ones_sb[:],
                                ats[pi][:, kc, :],
                                start=(kc == 0),
                                stop=(kc == 1),
                                skip_group_check=True,
                            )
                    rbc = rbc_pool.tile([P, 512], f32, name="rbc")
                    nc.vector.reciprocal_approx_fast(out=rbc[:], in_=den_ps[:])
                    nc.vector.tensor_tensor(
                        out=xoT[:, j, ssl],
                        in0=xo_ps[:],
                        in1=rbc[:],
                        op=mybir.AluOpType.mult,
                    )

                for sn in range(NSN):
                    for j in range(H // 2):
                        attn_pair(sn, j)
                    # output projection for this sn group (all heads now done)
                    for si in range(4):
                        sc = sn * 4 + si
                        for half in range(2):
                            ps = ps_out.tile([P, 512], f32, name="ps_o")
                            for c in range(NDC):
                                nc.tensor.matmul(
                                    ps[:],
                                    xoT[:, c, sc * P : (sc + 1) * P],
                                    wo_bf[:, c, half * 512 : (half + 1) * 512],
                                    start=(c == 0),
                                    stop=(c == NDC - 1),
                                )
                            osb = osb_pool.tile([P, 512], f32, name="osb")
                            nc.vector.tensor_tensor(
                                out=osb[:],
                                in0=ps[:],
                                in1=bo_bc[:, half * 512 : (half + 1) * 512],
                                op=mybir.AluOpType.add,
                            )
                            nc.sync.dma_start(
                                out=out_e[sc * P : (sc + 1) * P, half * 512 : (half + 1) * 512],
                                in_=osb[:],
                            )
            if debug:
                dbg_kpT = nc.declare_dram_parameter("dbg_kpT", [P, H // 2, LK], f32, isOutput=True)
                dbg_vp = nc.declare_dram_parameter("dbg_vp", [P, H, 2, HD], f32, isOutput=True)
                dbg_qT = nc.declare_dram_parameter("dbg_qT", [P, NDC, SL], f32, isOutput=True)
                dbg_xoT = nc.declare_dram_parameter("dbg_xoT", [P, NDC, SL], f32, isOutput=True)
                nc.gpsimd.dma_start(out=dbg_kpT[:], in_=kpT[:])
                nc.gpsimd.dma_start(out=dbg_vp[:], in_=vp_sb2[:])
                nc.gpsimd.dma_start(out=dbg_qT[:], in_=qT[:])
                nc.gpsimd.dma_start(out=dbg_xoT[:], in_=xoT[:])
            free_wo()
            free_xoT()
            free_vp()
            free_kpT()
            free_qT_hi()
            free_wq()
            free_qT_lo()
            free_xT()

    nc.compile()
    return nc


_cache = {}


def _get_nc(include_biases: bool):
    if include_biases not in _cache:
        _cache[include_biases] = _build(include_biases)
    return _cache[include_biases]


def kernel(**inputs) -> np.ndarray:
    X = np.asarray(inputs["X"], np.float32)
    mask = np.asarray(inputs["mask"], np.float32)
    E = np.asarray(inputs["E"], np.float32)
    Ws = {k: np.asarray(inputs[k], np.float32) for k in ("Wq", "Wk", "Wv", "Wo")}
    bs = {k: np.asarray(inputs[k], np.float32) for k in ("bq", "bk", "bv", "bo")}

    include_biases = bool(np.any(bs["bk"]) or np.any(bs["bv"]))
    nc = _get_nc(include_biases)

    in_maps = []
    for c in range(8):
        b, half = c // 2, c % 2
        sl = slice(half * SL, (half + 1) * SL)
        in_maps.append(
            {
                "X": np.ascontiguousarray(X[b, sl, :]),
                "mask": np.ascontiguousarray(mask[b, sl]),
                "Wq": Ws["Wq"], "bq": bs["bq"],
                "Wk": Ws["Wk"], "bk": bs["bk"],
                "Wv": Ws["Wv"], "bv": bs["bv"],
                "E": np.ascontiguousarray(E[:, :, sl]),
                "Wo": Ws["Wo"], "bo": bs["bo"],
            }
        )

    res = bass_utils.run_bass_kernel_spmd(nc, in_maps, core_ids=list(range(8)))
    out = np.empty((B, S, D), np.float32)
    for c in range(8):
        b, half = c // 2, c % 2
        out[b, half * SL : (half + 1) * SL, :] = res.results[c]["out"]
    return out

